# revision 30
# baseline (speedup 1.0000x reference)
"""AdaptivePolyphaseSampling kernel for 8 TRN2 NeuronCores.

Reference semantics (STRIDE=2, P_NORM=2):
  x: [16, 96, 256, 256] f32
  poly[(i,j)] = x[:, :, i::2, j::2]           (4 components)
  norms[(i,j), b] = sum(poly^2 over C,H',W')  (monotone in p-norm)
  idx[b] = argmax over the 4 components
  out[b] = poly[idx[b], b]  -> [16, 96, 128, 128]

Sharding: pure data parallel over batch; 2 samples per core, no
communication.

Per-core algorithm (full-sample SBUF residency):
  One sample (96ch x 512 f32/partition = 192KiB/partition) fits in SBUF
  (~208KiB usable). Stream the sample into 13 resident chunk tiles
  (11x8ch + 2x4ch tail chunks that shorten the argmax dependency tail);
  as each chunk lands, square+accumulate the 4 phase partial sums
  (3 phases fused on the ACT engine via activation(Square, accum_out),
  1 phase on DVE as mult+reduce into PSUM scratch). Then finalize the
  argmax fully on-chip (partition_all_reduce + pairwise-max compares),
  load the winning (i, j) into registers, and copy the winning
  polyphase component straight out of the resident tiles with
  dynamic-offset access patterns. x is read from HBM exactly once; the
  total DRAM traffic is the 50.3MB read + 12.6MB write floor.
  Sample 1 reuses the chunk slots; its loads chase sample 0's
  selects chunk-by-chunk (Tile WAR deps), overlapping the store of s0
  with the load of s1. An explicit dep keeps s0's selects ahead of
  s1's DVE squares so the select stream is never convoyed.
"""

import numpy as np

import concourse.bass as bass
import concourse.bacc as bacc
import concourse.bass_isa as bass_isa
import concourse.mybir as mybir
import concourse.tile as tile
from concourse.bass import ds
from concourse.bass_utils import run_bass_kernel_spmd

N_CORES = 8
B = 16
C = 96
H = 256
W = 256
H2 = H // 2
W2 = W // 2
BPC = B // N_CORES  # samples per core

F32 = mybir.dt.float32
I32 = mybir.dt.int32

NCB = 8            # channels per resident chunk tile
# 11 big chunks + 2 small tail chunks: the final load->square->argmax
# dependency tail is halved
CHUNKS = [(k * NCB, NCB) for k in range(11)] + [(88, 4), (92, 4)]
NCHUNK = len(CHUNKS)  # 13
NCS = 4            # channels per select / out-DMA call (mid-pipeline)

# E[x^2]=1 for randn input; subtracting the expected per-sample-partition
# sum before the cross-partition reduce keeps the accumulation near zero
# so fp32 rounding cannot flip the argmax.
EXP_PHASE_PART = float(C * H2 * W2 / 128)  # 12288 per partition per phase


def build_kernel():
    nc = bacc.Bacc("TRN2", target_bir_lowering=False, debug=False,
                   num_devices=N_CORES)
    x_ext = nc.dram_tensor("x", [BPC, C, H, W], F32, kind="ExternalInput")
    out_ext = nc.dram_tensor("out", [BPC, C, H2, W2], F32, kind="ExternalOutput")

    with tile.TileContext(nc) as tc:
        _emit(tc, nc, x_ext, out_ext)
    nc.compile()
    return nc


def _emit(tc, nc, x_ext, out_ext):
    import contextlib
    ctx = contextlib.ExitStack()
    with ctx:
        p_res = ctx.enter_context(tc.tile_pool(name="p_res", bufs=11))
        p_res2 = ctx.enter_context(tc.tile_pool(name="p_res2", bufs=2))
        p_psum = ctx.enter_context(
            tc.tile_pool(name="p_psum", bufs=2, space="PSUM"))
        p_acc = ctx.enter_context(tc.tile_pool(name="p_acc", bufs=1))
        p_small = ctx.enter_context(tc.tile_pool(name="p_small", bufs=1))
        p_out = ctx.enter_context(tc.tile_pool(name="p_out", bufs=2))

        # partials: per sample, [phase, chunk] so one strided reduce
        # finalizes all 4 phases
        # phase-2 goes to DVE for the last big chunks so ACT has tail
        # slack at the sample boundary
        DVE_PH2 = {8, 9, 10}
        ACT_PH2 = [k for k in range(NCHUNK) if k not in DVE_PH2]
        # separate accumulator tiles per engine (a shared tile false-
        # serializes ACT accum-writes against DVE reduce-writes)
        A_COLS = 2 * NCHUNK + len(ACT_PH2)
        HALVES = [max(1, c[1] // NCS) for c in CHUNKS]
        DVE3_COL0 = [sum(HALVES[:k]) for k in range(NCHUNK)]
        NDVE3 = sum(HALVES)
        D_COLS = 2 * len(DVE_PH2) + NDVE3
        acc_act = p_acc.tile([128, BPC * A_COLS], F32, tag="acc_act")
        acc_dve = p_acc.tile([128, BPC * D_COLS], F32, tag="acc_dve")

        from concourse.ordered_set import OrderedSet
        from concourse.tile_rust import add_dep_helper
        veng = OrderedSet([mybir.EngineType.DVE])

        def load_chunk(s, k):
            c0, nch = CHUNKS[k]
            pool, tg = (p_res, "res") if nch == NCB else (p_res2, "res2")
            t = pool.tile([128, nch * 2 * W], F32, tag=tg)
            src = x_ext[s, c0:c0 + nch].rearrange(
                "c (h2 i) w -> h2 c (i w)", i=2)
            tv = t[:].rearrange("p (c iw) -> p c iw", c=nch)
            nc.sync.dma_start(tv, src)
            return t

        def squares_chunk(s, k, t):
            """Square+accumulate the 4 phases of chunk k; returns the first
            DVE instruction (for explicit ordering)."""
            nch = CHUNKS[k][1]
            tp = t[:].rearrange("p (c i w2 j) -> p c i w2 j",
                                c=nch, i=2, w2=W2, j=2)
            # phases 0,1 (+2 for ACT_PH2 chunks): ACT fused
            # square+accumulate, whole chunk
            phases = [(0, 0), (0, 1)]
            if k not in DVE_PH2:
                phases.append((1, 0))
            for ph, (pi, pj) in enumerate(phases):
                if ph < 2:
                    col = s * A_COLS + ph * NCHUNK + k
                else:
                    col = s * A_COLS + 2 * NCHUNK + ACT_PH2.index(k)
                sq_a = p_psum.tile([128, NCB * W2], F32, tag="sq_act")
                sqv = sq_a[:, 0:nch * W2].rearrange(
                    "p (c w2) -> p c w2", c=nch)
                nc.scalar.activation(
                    sqv, tp[:, :, pi, :, pj],
                    mybir.ActivationFunctionType.Square,
                    accum_out=acc_act[:, col:col + 1])

            # DVE phases (always 3; plus 2 for DVE_PH2 chunks):
            # mult -> PSUM, dense reduce, in 4ch halves
            dve_phases = [((1, 1), s * D_COLS + 2 * len(DVE_PH2)
                           + DVE3_COL0[k])]
            if k in DVE_PH2:
                di = sorted(DVE_PH2).index(k)
                dve_phases.append(((1, 0), s * D_COLS + 2 * di))
            tt = None
            for (pi, pj), col0 in dve_phases:
                for half in range(max(1, nch // NCS)):
                    cs = half * NCS
                    sq_d = p_psum.tile([128, NCS * W2], F32, tag="sq_dve")
                    t2 = nc.vector.tensor_tensor(
                        out=sq_d[:].rearrange("p (c w2) -> p c w2", c=NCS),
                        in0=tp[:, cs:cs + NCS, pi, :, pj],
                        in1=tp[:, cs:cs + NCS, pi, :, pj],
                        op=mybir.AluOpType.mult)
                    if tt is None:
                        tt = t2
                    nc.vector.reduce_sum(
                        acc_dve[:, col0 + half:col0 + half + 1], sq_d[:],
                        axis=mybir.AxisListType.X)
            return tt

        def argmax(s):
            sums4 = p_small.tile([128, 4], F32, tag=f"sums4_{s}")
            a0 = acc_act[:, s * A_COLS:s * A_COLS + 2 * NCHUNK].rearrange(
                "p (f k) -> p f k", k=NCHUNK, f=2)
            nc.vector.reduce_sum(sums4[:, 0:2], a0,
                                 axis=mybir.AxisListType.X)
            a2 = acc_act[:, s * A_COLS + 2 * NCHUNK:(s + 1) * A_COLS]
            nc.vector.reduce_sum(sums4[:, 2:3], a2,
                                 axis=mybir.AxisListType.X)
            d3 = acc_dve[:, s * D_COLS + 2 * len(DVE_PH2):(s + 1) * D_COLS]
            nc.vector.reduce_sum(sums4[:, 3:4], d3,
                                 axis=mybir.AxisListType.X)
            ph2b = p_small.tile([128, 1], F32, tag=f"ph2b_{s}")
            d2 = acc_dve[:, s * D_COLS:s * D_COLS + 2 * len(DVE_PH2)]
            nc.vector.reduce_sum(ph2b[:, 0:1], d2,
                                 axis=mybir.AxisListType.X)
            nc.vector.tensor_tensor(
                out=sums4[:, 2:3], in0=sums4[:, 2:3], in1=ph2b[:, 0:1],
                op=mybir.AluOpType.add)
            # center before cross-partition accumulation (fp32 argmax safety)
            nc.vector.tensor_scalar(
                sums4[:], sums4[:], EXP_PHASE_PART, None,
                mybir.AluOpType.subtract)
            red4 = p_small.tile([128, 4], F32, tag=f"red4_{s}")
            nc.gpsimd.partition_all_reduce(
                red4[:], sums4[:], channels=128,
                reduce_op=bass_isa.ReduceOp.add)

            # pairmax trick: i = (max(s2,s3) > max(s0,s1)),
            # j = (max(s1,s3) > max(s0,s2)); exact ties are measure-zero
            pmx = p_small.tile([1, 4], F32, tag=f"pmx_{s}")
            r4i = red4[0:1, 0:4].rearrange("p (i j) -> p i j", i=2, j=2)
            nc.vector.reduce_max(pmx[0:1, 0:2], r4i,
                                 axis=mybir.AxisListType.X)
            r4j = red4[0:1, 0:4].rearrange("p (i j) -> p j i", i=2, j=2)
            nc.vector.reduce_max(pmx[0:1, 2:4], r4j,
                                 axis=mybir.AxisListType.X)
            ij_f = p_small.tile([1, 2], F32, tag=f"ij_f_{s}")
            pv = pmx[0:1, 0:4].rearrange("p (a b) -> p a b", a=2, b=2)
            nc.vector.tensor_tensor(
                out=ij_f[0:1, :], in0=pv[:, :, 1], in1=pv[:, :, 0],
                op=mybir.AluOpType.is_gt)
            ij_i = p_small.tile([1, 2], I32, tag=f"ij_i_{s}")
            nc.vector.tensor_copy(ij_i[0:1, :], ij_f[0:1, :])
            i_val = nc.values_load(ij_i[0:1, 0:1], engines=veng,
                                   min_val=0, max_val=1,
                                   skip_runtime_bounds_check=True)
            j_val = nc.values_load(ij_i[0:1, 1:2], engines=veng,
                                   min_val=0, max_val=1,
                                   skip_runtime_bounds_check=True)
            return i_val, j_val

        def select_chunk(s, k, t, i_val, j_val, ncs):
            c0, nch = CHUNKS[k]
            tp = t[:].rearrange("p (c i w2 j) -> p c i w2 j",
                                c=nch, i=2, w2=W2, j=2)
            sel = None
            ncs = min(ncs, nch)
            for half in range(nch // ncs):
                cs = half * ncs
                o = p_out.tile([128, NCB * W2], F32, tag="outt")
                src = tp[:, cs:cs + ncs, ds(i_val, 1), :, ds(j_val, 1)]
                ov = o[:, 0:ncs * W2].rearrange(
                    "p (c i w2 j) -> p c i w2 j", c=ncs, i=1, w2=W2, j=1)
                sel = nc.vector.tensor_copy(ov, src)
                dst = out_ext[s, c0 + cs:c0 + cs + ncs].rearrange(
                    "c h2 w2 -> h2 c w2")
                nc.sync.dma_start(
                    dst, o[:, 0:ncs * W2].rearrange(
                        "p (c w2) -> p c w2", c=ncs))
            return sel

        # sample 0: all loads first (ACT-ring kicks schedule ahead of
        # the ACT square stream), then squares
        tiles = [load_chunk(0, k) for k in range(NCHUNK)]
        for k in range(NCHUNK):
            squares_chunk(0, k, tiles[k])

        for s in range(BPC):
            i_val, j_val = argmax(s)
            # interleave: select/store chunk k of s, then load chunk k of
            # s+1 into the freed slot (same SP emission order)
            nxt = []
            last_sel = None
            ncs = NCS if s + 1 < BPC else NCB
            for k in range(NCHUNK):
                last_sel = select_chunk(s, k, tiles[k], i_val, j_val, ncs)
                if s + 1 < BPC:
                    nxt.append(load_chunk(s + 1, k))
            if s + 1 < BPC:
                first_tt = None
                for k in range(NCHUNK):
                    tt = squares_chunk(s + 1, k, nxt[k])
                    if first_tt is None:
                        first_tt = tt
                # keep sample-s selects ahead of sample-s+1 squares in the
                # DVE stream (scheduler would otherwise interleave and
                # convoy the selects behind stalled squares)
                add_dep_helper(last_sel.ins, first_tt.ins, sync=False,
                               reason="selects before next-sample squares")
                tiles = nxt


_NC = None


def _get_nc():
    global _NC
    if _NC is None:
        _NC = build_kernel()
    return _NC


def kernel(x: np.ndarray) -> np.ndarray:
    assert x.shape == (B, C, H, W) and x.dtype == np.float32
    nc = _get_nc()
    in_maps = [{"x": np.ascontiguousarray(x[c * BPC:(c + 1) * BPC])}
               for c in range(N_CORES)]
    res = run_bass_kernel_spmd(nc, in_maps, core_ids=list(range(N_CORES)))
    return np.concatenate([res.results[c]["out"] for c in range(N_CORES)],
                          axis=0)


# revision 31
# speedup vs baseline: 1.0585x; 1.0585x over previous
"""AdaptivePolyphaseSampling kernel for 8 TRN2 NeuronCores.

Reference semantics (STRIDE=2, P_NORM=2):
  x: [16, 96, 256, 256] f32
  poly[(i,j)] = x[:, :, i::2, j::2]           (4 components)
  norms[(i,j), b] = sum(poly^2 over C,H',W')  (monotone in p-norm)
  idx[b] = argmax over the 4 components
  out[b] = poly[idx[b], b]  -> [16, 96, 128, 128]

Sharding: pure data parallel over batch; 2 samples per core, no
communication.

Per-core algorithm (full-sample SBUF residency):
  One sample (96ch x 512 f32/partition = 192KiB/partition) fits in SBUF
  (~208KiB usable). Stream the sample into 13 resident chunk tiles
  (11x8ch + 2x4ch tail chunks that shorten the argmax dependency tail);
  as each chunk lands, square+accumulate the 4 phase partial sums
  (3 phases fused on the ACT engine via activation(Square, accum_out),
  1 phase on DVE as mult+reduce into PSUM scratch). Then finalize the
  argmax fully on-chip (partition_all_reduce + pairwise-max compares),
  load the winning (i, j) into registers, and copy the winning
  polyphase component straight out of the resident tiles with
  dynamic-offset access patterns. x is read from HBM exactly once; the
  total DRAM traffic is the 50.3MB read + 12.6MB write floor.
  Sample 1 reuses the chunk slots; its loads chase sample 0's
  selects chunk-by-chunk (Tile WAR deps), overlapping the store of s0
  with the load of s1. An explicit dep keeps s0's selects ahead of
  s1's DVE squares so the select stream is never convoyed.
"""

import numpy as np

import concourse.bass as bass
import concourse.bacc as bacc
import concourse.bass_isa as bass_isa
import concourse.mybir as mybir
import concourse.tile as tile
from concourse.bass import ds
from concourse.bass_utils import run_bass_kernel_spmd

N_CORES = 8
B = 16
C = 96
H = 256
W = 256
H2 = H // 2
W2 = W // 2
BPC = B // N_CORES  # samples per core

F32 = mybir.dt.float32
I32 = mybir.dt.int32

NCB = 8            # channels per resident chunk tile
# 11 big chunks + 2 small tail chunks: the final load->square->argmax
# dependency tail is halved
CHUNKS = [(k * NCB, NCB) for k in range(11)] + [(88, 4), (92, 4)]
NCHUNK = len(CHUNKS)  # 13
NCS = 4            # channels per select / out-DMA call (mid-pipeline)

# E[x^2]=1 for randn input; subtracting the expected per-sample-partition
# sum before the cross-partition reduce keeps the accumulation near zero
# so fp32 rounding cannot flip the argmax.
EXP_PHASE_PART = float(C * H2 * W2 / 128)  # 12288 per partition per phase


def build_kernel():
    nc = bacc.Bacc("TRN2", target_bir_lowering=False, debug=False,
                   num_devices=N_CORES)
    x_ext = nc.dram_tensor("x", [BPC, C, H, W], F32, kind="ExternalInput")
    out_ext = nc.dram_tensor("out", [BPC, C, H2, W2], F32, kind="ExternalOutput")

    with tile.TileContext(nc) as tc:
        _emit(tc, nc, x_ext, out_ext)
    nc.compile()
    return nc


def _emit(tc, nc, x_ext, out_ext):
    import contextlib
    ctx = contextlib.ExitStack()
    with ctx:
        p_res = ctx.enter_context(tc.tile_pool(name="p_res", bufs=11))
        p_res2 = ctx.enter_context(tc.tile_pool(name="p_res2", bufs=2))
        p_psum = ctx.enter_context(
            tc.tile_pool(name="p_psum", bufs=2, space="PSUM"))
        p_acc = ctx.enter_context(tc.tile_pool(name="p_acc", bufs=1))
        p_small = ctx.enter_context(tc.tile_pool(name="p_small", bufs=1))
        p_out = ctx.enter_context(tc.tile_pool(name="p_out", bufs=2))
        # DVE square scratch in SBUF: PSUM access on DVE measured ~3x
        # slower than expected (~3.1 cyc/elem on both the mult write and
        # the reduce read)
        p_sqd = ctx.enter_context(tc.tile_pool(name="p_sqd", bufs=2))

        # partials: per sample, [phase, chunk] so one strided reduce
        # finalizes all 4 phases
        # phase-2 goes to DVE for the last big chunks so ACT has tail
        # slack at the sample boundary
        DVE_PH2 = {8, 9, 10}
        ACT_PH2 = [k for k in range(NCHUNK) if k not in DVE_PH2]
        # separate accumulator tiles per engine (a shared tile false-
        # serializes ACT accum-writes against DVE reduce-writes)
        A_COLS = 2 * NCHUNK + len(ACT_PH2)
        HALVES = [max(1, c[1] // NCS) for c in CHUNKS]
        DVE3_COL0 = [sum(HALVES[:k]) for k in range(NCHUNK)]
        NDVE3 = sum(HALVES)
        D_COLS = 2 * len(DVE_PH2) + NDVE3
        acc_act = p_acc.tile([128, BPC * A_COLS], F32, tag="acc_act")
        acc_dve = p_acc.tile([128, BPC * D_COLS], F32, tag="acc_dve")

        from concourse.ordered_set import OrderedSet
        from concourse.tile_rust import add_dep_helper
        veng = OrderedSet([mybir.EngineType.DVE])

        def load_chunk(s, k):
            c0, nch = CHUNKS[k]
            pool, tg = (p_res, "res") if nch == NCB else (p_res2, "res2")
            t = pool.tile([128, nch * 2 * W], F32, tag=tg)
            src = x_ext[s, c0:c0 + nch].rearrange(
                "c (h2 i) w -> h2 c (i w)", i=2)
            tv = t[:].rearrange("p (c iw) -> p c iw", c=nch)
            nc.sync.dma_start(tv, src)
            return t

        def squares_chunk(s, k, t):
            """Square+accumulate the 4 phases of chunk k; returns the first
            DVE instruction (for explicit ordering)."""
            nch = CHUNKS[k][1]
            tp = t[:].rearrange("p (c i w2 j) -> p c i w2 j",
                                c=nch, i=2, w2=W2, j=2)
            # phases 0,1 (+2 for ACT_PH2 chunks): ACT fused
            # square+accumulate, whole chunk
            phases = [(0, 0), (0, 1)]
            if k not in DVE_PH2:
                phases.append((1, 0))
            for ph, (pi, pj) in enumerate(phases):
                if ph < 2:
                    col = s * A_COLS + ph * NCHUNK + k
                else:
                    col = s * A_COLS + 2 * NCHUNK + ACT_PH2.index(k)
                sq_a = p_psum.tile([128, NCB * W2], F32, tag="sq_act")
                sqv = sq_a[:, 0:nch * W2].rearrange(
                    "p (c w2) -> p c w2", c=nch)
                nc.scalar.activation(
                    sqv, tp[:, :, pi, :, pj],
                    mybir.ActivationFunctionType.Square,
                    accum_out=acc_act[:, col:col + 1])

            # DVE phases (always 3; plus 2 for DVE_PH2 chunks):
            # mult -> PSUM, dense reduce, in 4ch halves
            dve_phases = [((1, 1), s * D_COLS + 2 * len(DVE_PH2)
                           + DVE3_COL0[k])]
            if k in DVE_PH2:
                di = sorted(DVE_PH2).index(k)
                dve_phases.append(((1, 0), s * D_COLS + 2 * di))
            tt = None
            for (pi, pj), col0 in dve_phases:
                for half in range(max(1, nch // NCS)):
                    cs = half * NCS
                    sq_d = p_sqd.tile([128, NCS * W2], F32, tag="sq_dve")
                    t2 = nc.vector.tensor_tensor(
                        out=sq_d[:].rearrange("p (c w2) -> p c w2", c=NCS),
                        in0=tp[:, cs:cs + NCS, pi, :, pj],
                        in1=tp[:, cs:cs + NCS, pi, :, pj],
                        op=mybir.AluOpType.mult)
                    if tt is None:
                        tt = t2
                    nc.vector.reduce_sum(
                        acc_dve[:, col0 + half:col0 + half + 1], sq_d[:],
                        axis=mybir.AxisListType.X)
            return tt

        def argmax(s):
            sums4 = p_small.tile([128, 4], F32, tag=f"sums4_{s}")
            a0 = acc_act[:, s * A_COLS:s * A_COLS + 2 * NCHUNK].rearrange(
                "p (f k) -> p f k", k=NCHUNK, f=2)
            nc.vector.reduce_sum(sums4[:, 0:2], a0,
                                 axis=mybir.AxisListType.X)
            a2 = acc_act[:, s * A_COLS + 2 * NCHUNK:(s + 1) * A_COLS]
            nc.vector.reduce_sum(sums4[:, 2:3], a2,
                                 axis=mybir.AxisListType.X)
            d3 = acc_dve[:, s * D_COLS + 2 * len(DVE_PH2):(s + 1) * D_COLS]
            nc.vector.reduce_sum(sums4[:, 3:4], d3,
                                 axis=mybir.AxisListType.X)
            ph2b = p_small.tile([128, 1], F32, tag=f"ph2b_{s}")
            d2 = acc_dve[:, s * D_COLS:s * D_COLS + 2 * len(DVE_PH2)]
            nc.vector.reduce_sum(ph2b[:, 0:1], d2,
                                 axis=mybir.AxisListType.X)
            nc.vector.tensor_tensor(
                out=sums4[:, 2:3], in0=sums4[:, 2:3], in1=ph2b[:, 0:1],
                op=mybir.AluOpType.add)
            # center before cross-partition accumulation (fp32 argmax safety)
            nc.vector.tensor_scalar(
                sums4[:], sums4[:], EXP_PHASE_PART, None,
                mybir.AluOpType.subtract)
            red4 = p_small.tile([128, 4], F32, tag=f"red4_{s}")
            nc.gpsimd.partition_all_reduce(
                red4[:], sums4[:], channels=128,
                reduce_op=bass_isa.ReduceOp.add)

            # pairmax trick: i = (max(s2,s3) > max(s0,s1)),
            # j = (max(s1,s3) > max(s0,s2)); exact ties are measure-zero
            pmx = p_small.tile([1, 4], F32, tag=f"pmx_{s}")
            r4i = red4[0:1, 0:4].rearrange("p (i j) -> p i j", i=2, j=2)
            nc.vector.reduce_max(pmx[0:1, 0:2], r4i,
                                 axis=mybir.AxisListType.X)
            r4j = red4[0:1, 0:4].rearrange("p (i j) -> p j i", i=2, j=2)
            nc.vector.reduce_max(pmx[0:1, 2:4], r4j,
                                 axis=mybir.AxisListType.X)
            ij_f = p_small.tile([1, 2], F32, tag=f"ij_f_{s}")
            pv = pmx[0:1, 0:4].rearrange("p (a b) -> p a b", a=2, b=2)
            nc.vector.tensor_tensor(
                out=ij_f[0:1, :], in0=pv[:, :, 1], in1=pv[:, :, 0],
                op=mybir.AluOpType.is_gt)
            ij_i = p_small.tile([1, 2], I32, tag=f"ij_i_{s}")
            nc.vector.tensor_copy(ij_i[0:1, :], ij_f[0:1, :])
            i_val = nc.values_load(ij_i[0:1, 0:1], engines=veng,
                                   min_val=0, max_val=1,
                                   skip_runtime_bounds_check=True)
            j_val = nc.values_load(ij_i[0:1, 1:2], engines=veng,
                                   min_val=0, max_val=1,
                                   skip_runtime_bounds_check=True)
            return i_val, j_val

        def select_chunk(s, k, t, i_val, j_val, ncs):
            c0, nch = CHUNKS[k]
            tp = t[:].rearrange("p (c i w2 j) -> p c i w2 j",
                                c=nch, i=2, w2=W2, j=2)
            sel = None
            ncs = min(ncs, nch)
            for half in range(nch // ncs):
                cs = half * ncs
                o = p_out.tile([128, NCB * W2], F32, tag="outt")
                src = tp[:, cs:cs + ncs, ds(i_val, 1), :, ds(j_val, 1)]
                ov = o[:, 0:ncs * W2].rearrange(
                    "p (c i w2 j) -> p c i w2 j", c=ncs, i=1, w2=W2, j=1)
                sel = nc.vector.tensor_copy(ov, src)
                dst = out_ext[s, c0 + cs:c0 + cs + ncs].rearrange(
                    "c h2 w2 -> h2 c w2")
                nc.sync.dma_start(
                    dst, o[:, 0:ncs * W2].rearrange(
                        "p (c w2) -> p c w2", c=ncs))
            return sel

        # sample 0: all loads first (ACT-ring kicks schedule ahead of
        # the ACT square stream), then squares
        tiles = [load_chunk(0, k) for k in range(NCHUNK)]
        for k in range(NCHUNK):
            squares_chunk(0, k, tiles[k])

        for s in range(BPC):
            i_val, j_val = argmax(s)
            # interleave: select/store chunk k of s, then load chunk k of
            # s+1 into the freed slot (same SP emission order)
            nxt = []
            last_sel = None
            ncs = NCS if s + 1 < BPC else NCB
            for k in range(NCHUNK):
                last_sel = select_chunk(s, k, tiles[k], i_val, j_val, ncs)
                if s + 1 < BPC:
                    nxt.append(load_chunk(s + 1, k))
            if s + 1 < BPC:
                first_tt = None
                for k in range(NCHUNK):
                    tt = squares_chunk(s + 1, k, nxt[k])
                    if first_tt is None:
                        first_tt = tt
                # keep sample-s selects ahead of sample-s+1 squares in the
                # DVE stream (scheduler would otherwise interleave and
                # convoy the selects behind stalled squares)
                add_dep_helper(last_sel.ins, first_tt.ins, sync=False,
                               reason="selects before next-sample squares")
                tiles = nxt


_NC = None


def _get_nc():
    global _NC
    if _NC is None:
        _NC = build_kernel()
    return _NC


def kernel(x: np.ndarray) -> np.ndarray:
    assert x.shape == (B, C, H, W) and x.dtype == np.float32
    nc = _get_nc()
    in_maps = [{"x": np.ascontiguousarray(x[c * BPC:(c + 1) * BPC])}
               for c in range(N_CORES)]
    res = run_bass_kernel_spmd(nc, in_maps, core_ids=list(range(N_CORES)))
    return np.concatenate([res.results[c]["out"] for c in range(N_CORES)],
                          axis=0)


# revision 32
# speedup vs baseline: 1.1287x; 1.0663x over previous
"""AdaptivePolyphaseSampling kernel for 8 TRN2 NeuronCores.

Reference semantics (STRIDE=2, P_NORM=2):
  x: [16, 96, 256, 256] f32
  poly[(i,j)] = x[:, :, i::2, j::2]           (4 components)
  norms[(i,j), b] = sum(poly^2 over C,H',W')  (monotone in p-norm)
  idx[b] = argmax over the 4 components
  out[b] = poly[idx[b], b]  -> [16, 96, 128, 128]

Sharding: pure data parallel over batch; 2 samples per core, no
communication.

Per-core algorithm (full-sample SBUF residency):
  One sample (96ch x 512 f32/partition = 192KiB/partition) fits in SBUF
  (~208KiB usable). Stream the sample into 13 resident chunk tiles
  (11x8ch + 2x4ch tail chunks that shorten the argmax dependency tail);
  as each chunk lands, square+accumulate the 4 phase partial sums
  (3 phases fused on the ACT engine via activation(Square, accum_out),
  1 phase on DVE as mult+reduce into PSUM scratch). Then finalize the
  argmax fully on-chip (partition_all_reduce + pairwise-max compares),
  load the winning (i, j) into registers, and copy the winning
  polyphase component straight out of the resident tiles with
  dynamic-offset access patterns. x is read from HBM exactly once; the
  total DRAM traffic is the 50.3MB read + 12.6MB write floor.
  Sample 1 reuses the chunk slots; its loads chase sample 0's
  selects chunk-by-chunk (Tile WAR deps), overlapping the store of s0
  with the load of s1. An explicit dep keeps s0's selects ahead of
  s1's DVE squares so the select stream is never convoyed.
"""

import numpy as np

import concourse.bass as bass
import concourse.bacc as bacc
import concourse.bass_isa as bass_isa
import concourse.mybir as mybir
import concourse.tile as tile
from concourse.bass import ds
from concourse.bass_utils import run_bass_kernel_spmd

N_CORES = 8
B = 16
C = 96
H = 256
W = 256
H2 = H // 2
W2 = W // 2
BPC = B // N_CORES  # samples per core

F32 = mybir.dt.float32
I32 = mybir.dt.int32

NCB = 8            # channels per resident chunk tile
# 11 big chunks + 2 small tail chunks: the final load->square->argmax
# dependency tail is halved
CHUNKS = [(k * NCB, NCB) for k in range(11)] + [(88, 4), (92, 4)]
NCHUNK = len(CHUNKS)  # 13
NCS = 4            # channels per select / out-DMA call (mid-pipeline)

# E[x^2]=1 for randn input; subtracting the expected per-sample-partition
# sum before the cross-partition reduce keeps the accumulation near zero
# so fp32 rounding cannot flip the argmax.
EXP_PHASE_PART = float(C * H2 * W2 / 128)  # 12288 per partition per phase


def build_kernel():
    nc = bacc.Bacc("TRN2", target_bir_lowering=False, debug=False,
                   num_devices=N_CORES)
    x_ext = nc.dram_tensor("x", [BPC, C, H, W], F32, kind="ExternalInput")
    out_ext = nc.dram_tensor("out", [BPC, C, H2, W2], F32, kind="ExternalOutput")

    with tile.TileContext(nc) as tc:
        _emit(tc, nc, x_ext, out_ext)
    nc.compile()
    return nc


def _emit(tc, nc, x_ext, out_ext):
    import contextlib
    ctx = contextlib.ExitStack()
    with ctx:
        p_res = ctx.enter_context(tc.tile_pool(name="p_res", bufs=11))
        p_res2 = ctx.enter_context(tc.tile_pool(name="p_res2", bufs=2))
        p_psum = ctx.enter_context(
            tc.tile_pool(name="p_psum", bufs=2, space="PSUM"))
        p_acc = ctx.enter_context(tc.tile_pool(name="p_acc", bufs=1))
        p_small = ctx.enter_context(tc.tile_pool(name="p_small", bufs=1))
        p_out = ctx.enter_context(tc.tile_pool(name="p_out", bufs=2))

        # partials: per sample, [phase, chunk] so one strided reduce
        # finalizes all 4 phases
        # phase-2 goes to DVE for the last big chunks so ACT has tail
        # slack at the sample boundary
        DVE_PH2 = {8, 9, 10}
        ACT_PH2 = [k for k in range(NCHUNK) if k not in DVE_PH2]
        # separate accumulator tiles per engine (a shared tile false-
        # serializes ACT accum-writes against DVE reduce-writes)
        A_COLS = 2 * NCHUNK + len(ACT_PH2)
        HALVES = [max(1, c[1] // NCS) for c in CHUNKS]
        DVE3_COL0 = [sum(HALVES[:k]) for k in range(NCHUNK)]
        NDVE3 = sum(HALVES)
        D_COLS = 2 * len(DVE_PH2) + NDVE3
        acc_act = p_acc.tile([128, BPC * A_COLS], F32, tag="acc_act")
        acc_dve = p_acc.tile([128, BPC * D_COLS], F32, tag="acc_dve")

        from concourse.ordered_set import OrderedSet
        from concourse.tile_rust import add_dep_helper
        veng = OrderedSet([mybir.EngineType.DVE])

        def load_chunk(s, k):
            c0, nch = CHUNKS[k]
            pool, tg = (p_res, "res") if nch == NCB else (p_res2, "res2")
            t = pool.tile([128, nch * 2 * W], F32, tag=tg)
            src = x_ext[s, c0:c0 + nch].rearrange(
                "c (h2 i) w -> h2 c (i w)", i=2)
            tv = t[:].rearrange("p (c iw) -> p c iw", c=nch)
            nc.sync.dma_start(tv, src)
            return t

        def squares_chunk(s, k, t):
            """Square+accumulate the 4 phases of chunk k; returns the first
            DVE instruction (for explicit ordering)."""
            nch = CHUNKS[k][1]
            tp = t[:].rearrange("p (c i w2 j) -> p c i w2 j",
                                c=nch, i=2, w2=W2, j=2)
            # phases 0,1 (+2 for ACT_PH2 chunks): ACT fused
            # square+accumulate, whole chunk
            phases = [(0, 0), (0, 1)]
            if k not in DVE_PH2:
                phases.append((1, 0))
            for ph, (pi, pj) in enumerate(phases):
                if ph < 2:
                    col = s * A_COLS + ph * NCHUNK + k
                else:
                    col = s * A_COLS + 2 * NCHUNK + ACT_PH2.index(k)
                sq_a = p_psum.tile([128, NCB * W2], F32, tag="sq_act")
                sqv = sq_a[:, 0:nch * W2].rearrange(
                    "p (c w2) -> p c w2", c=nch)
                nc.scalar.activation(
                    sqv, tp[:, :, pi, :, pj],
                    mybir.ActivationFunctionType.Square,
                    accum_out=acc_act[:, col:col + 1])

            # DVE phases (always 3; plus 2 for DVE_PH2 chunks):
            # mult -> PSUM, dense reduce, in 4ch halves
            dve_phases = [((1, 1), s * D_COLS + 2 * len(DVE_PH2)
                           + DVE3_COL0[k])]
            if k in DVE_PH2:
                di = sorted(DVE_PH2).index(k)
                dve_phases.append(((1, 0), s * D_COLS + 2 * di))
            tt = None
            for (pi, pj), col0 in dve_phases:
                for half in range(max(1, nch // NCS)):
                    cs = half * NCS
                    sq_d = p_psum.tile([128, NCS * W2], F32, tag="sq_dve")
                    t2 = nc.vector.tensor_tensor(
                        out=sq_d[:].rearrange("p (c w2) -> p c w2", c=NCS),
                        in0=tp[:, cs:cs + NCS, pi, :, pj],
                        in1=tp[:, cs:cs + NCS, pi, :, pj],
                        op=mybir.AluOpType.mult)
                    if tt is None:
                        tt = t2
                    nc.vector.reduce_sum(
                        acc_dve[:, col0 + half:col0 + half + 1], sq_d[:],
                        axis=mybir.AxisListType.X)
            return tt

        def argmax(s):
            sums4 = p_small.tile([128, 4], F32, tag=f"sums4_{s}")
            a0 = acc_act[:, s * A_COLS:s * A_COLS + 2 * NCHUNK].rearrange(
                "p (f k) -> p f k", k=NCHUNK, f=2)
            nc.vector.reduce_sum(sums4[:, 0:2], a0,
                                 axis=mybir.AxisListType.X)
            a2 = acc_act[:, s * A_COLS + 2 * NCHUNK:(s + 1) * A_COLS]
            nc.vector.reduce_sum(sums4[:, 2:3], a2,
                                 axis=mybir.AxisListType.X)
            d3 = acc_dve[:, s * D_COLS + 2 * len(DVE_PH2):(s + 1) * D_COLS]
            nc.vector.reduce_sum(sums4[:, 3:4], d3,
                                 axis=mybir.AxisListType.X)
            ph2b = p_small.tile([128, 1], F32, tag=f"ph2b_{s}")
            d2 = acc_dve[:, s * D_COLS:s * D_COLS + 2 * len(DVE_PH2)]
            nc.vector.reduce_sum(ph2b[:, 0:1], d2,
                                 axis=mybir.AxisListType.X)
            nc.vector.tensor_tensor(
                out=sums4[:, 2:3], in0=sums4[:, 2:3], in1=ph2b[:, 0:1],
                op=mybir.AluOpType.add)
            # center before cross-partition accumulation (fp32 argmax safety)
            nc.vector.tensor_scalar(
                sums4[:], sums4[:], EXP_PHASE_PART, None,
                mybir.AluOpType.subtract)
            red4 = p_small.tile([128, 4], F32, tag=f"red4_{s}")
            nc.gpsimd.partition_all_reduce(
                red4[:], sums4[:], channels=128,
                reduce_op=bass_isa.ReduceOp.add)

            # pairmax trick: i = (max(s2,s3) > max(s0,s1)),
            # j = (max(s1,s3) > max(s0,s2)); exact ties are measure-zero
            pmx = p_small.tile([1, 4], F32, tag=f"pmx_{s}")
            r4i = red4[0:1, 0:4].rearrange("p (i j) -> p i j", i=2, j=2)
            nc.vector.reduce_max(pmx[0:1, 0:2], r4i,
                                 axis=mybir.AxisListType.X)
            r4j = red4[0:1, 0:4].rearrange("p (i j) -> p j i", i=2, j=2)
            nc.vector.reduce_max(pmx[0:1, 2:4], r4j,
                                 axis=mybir.AxisListType.X)
            ij_f = p_small.tile([1, 2], F32, tag=f"ij_f_{s}")
            pv = pmx[0:1, 0:4].rearrange("p (a b) -> p a b", a=2, b=2)
            nc.vector.tensor_tensor(
                out=ij_f[0:1, :], in0=pv[:, :, 1], in1=pv[:, :, 0],
                op=mybir.AluOpType.is_gt)
            ij_i = p_small.tile([1, 2], I32, tag=f"ij_i_{s}")
            nc.vector.tensor_copy(ij_i[0:1, :], ij_f[0:1, :])
            i_val = nc.values_load(ij_i[0:1, 0:1], engines=veng,
                                   min_val=0, max_val=1,
                                   skip_runtime_bounds_check=True)
            j_val = nc.values_load(ij_i[0:1, 1:2], engines=veng,
                                   min_val=0, max_val=1,
                                   skip_runtime_bounds_check=True)
            return i_val, j_val

        def select_chunk(s, k, t, i_val, j_val, ncs):
            c0, nch = CHUNKS[k]
            tp = t[:].rearrange("p (c i w2 j) -> p c i w2 j",
                                c=nch, i=2, w2=W2, j=2)
            sel = None
            ncs = min(ncs, nch)
            for half in range(nch // ncs):
                cs = half * ncs
                o = p_out.tile([128, NCB * W2], F32, tag="outt")
                src = tp[:, cs:cs + ncs, ds(i_val, 1), :, ds(j_val, 1)]
                ov = o[:, 0:ncs * W2].rearrange(
                    "p (c i w2 j) -> p c i w2 j", c=ncs, i=1, w2=W2, j=1)
                sel = nc.vector.tensor_copy(ov, src)
                dst = out_ext[s, c0 + cs:c0 + cs + ncs].rearrange(
                    "c h2 w2 -> h2 c w2")
                nc.sync.dma_start(
                    dst, o[:, 0:ncs * W2].rearrange(
                        "p (c w2) -> p c w2", c=ncs))
            return sel

        # sample 0: all loads first (ACT-ring kicks schedule ahead of
        # the ACT square stream), then squares
        tiles = [load_chunk(0, k) for k in range(NCHUNK)]
        for k in range(NCHUNK):
            squares_chunk(0, k, tiles[k])

        for s in range(BPC):
            i_val, j_val = argmax(s)
            # interleave: select/store chunk k of s, then load chunk k of
            # s+1 into the freed slot (same SP emission order)
            nxt = []
            last_sel = None
            ncs = NCS if s + 1 < BPC else NCB
            for k in range(NCHUNK):
                last_sel = select_chunk(s, k, tiles[k], i_val, j_val, ncs)
                if s + 1 < BPC:
                    nxt.append(load_chunk(s + 1, k))
            if s + 1 < BPC:
                first_tt = None
                for k in range(NCHUNK):
                    tt = squares_chunk(s + 1, k, nxt[k])
                    if first_tt is None:
                        first_tt = tt
                # keep sample-s selects ahead of sample-s+1 squares in the
                # DVE stream (scheduler would otherwise interleave and
                # convoy the selects behind stalled squares)
                add_dep_helper(last_sel.ins, first_tt.ins, sync=False,
                               reason="selects before next-sample squares")
                tiles = nxt


_NC = None


def _get_nc():
    global _NC
    if _NC is None:
        _NC = build_kernel()
    return _NC


def kernel(x: np.ndarray) -> np.ndarray:
    assert x.shape == (B, C, H, W) and x.dtype == np.float32
    nc = _get_nc()
    in_maps = [{"x": np.ascontiguousarray(x[c * BPC:(c + 1) * BPC])}
               for c in range(N_CORES)]
    res = run_bass_kernel_spmd(nc, in_maps, core_ids=list(range(N_CORES)))
    return np.concatenate([res.results[c]["out"] for c in range(N_CORES)],
                          axis=0)
